# revision 14
# baseline (speedup 1.0000x reference)
"""BWGNN (beta-wavelet GNN) Trainium2 kernel, 8-core SPMD.

Math: out_i = sqrt(d) * sum_k theta[i][k] * g_k, where g_0 = d^-1/2 * h,
g_{k+1} = g_k - d^-1 * segsum_dst(g_k[src]), h = leaky_relu(x @ W1 + b1).
All 5 filters are polynomials of the same propagation, so only 6 SpMM
rounds are needed (vs 30 in the naive formulation).

Sharding: nodes are block-sharded across 8 cores by dst. Each round a core
gathers source rows from an AllGather'd full table (rows packed as
[bf16(hi) | bf16(x-hi)] = 256B so one 256B dma_gather element carries
~f32 precision), segment-sums per 128-dst tile via one-hot matmuls on the
tensor engine (f32 PSUM accumulation), and updates its f32 state.
"""
import os
import sys
from math import comb, gamma

import numpy as np
import ml_dtypes

for _p in ('/opt/trn_rl_repo', os.path.expanduser('~/.axon_site/_ro/trn_rl_repo')):
    if os.path.isdir(_p) and _p not in sys.path:
        sys.path.append(_p)

import concourse.bass as bass
import concourse.bacc as bacc
import concourse.tile as tile
from concourse import bass_utils, mybir
from concourse.alu_op_type import AluOpType

F = 64          # hidden feature dim
FIN = 128       # input feature dim
KPOW = 6        # propagation rounds (powers 1..6; 7 snapshots incl. g_0)
NFILT = 5

f32 = mybir.dt.float32
bf16 = mybir.dt.bfloat16
i16 = mybir.dt.int16


def _thetas(d=4):
    c, off = 1.4, 2
    th = []
    for i in range(off, d + 1 + off):
        B = gamma(i + 1) * gamma(d + 1 - i + off) / gamma(d + 2 + off)
        m = d - i + off
        coeff = np.zeros(d + 1 + off, dtype=np.float64)
        for j in range(m + 1):
            coeff[i + j] = comb(m, j) * ((-1.0) ** j) / (c ** (i + j)) / (c * B)
        th.append(coeff)
    return np.array(th)  # [5, 7]


def _preprocess(src, dst, N, C, bucket_rows):
    """Host-side graph preprocessing -> per-core index tensors + schedule."""
    NLOC = N // C
    TILES = (NLOC + 127) // 128
    NPAD = TILES * 128
    TGRP = next(c for c in (7, 8, 6, 5, 4, 3, 2, 1) if TILES % c == 0)
    NGRP = TILES // TGRP

    deg = np.bincount(src, minlength=N).astype(np.float32)
    d = np.maximum(deg, 1.0)

    pos = (src // NLOC) * NPAD + (src % NLOC)       # padded global row of src
    bkt = pos // bucket_rows
    idxv = pos % bucket_rows
    NB = int((NPAD * C + bucket_rows - 1) // bucket_rows)
    assert bucket_rows <= 32768

    core = dst // NLOC
    dl = dst % NLOC
    tl = dl // 128
    pdst = dl % 128

    # per (core, tile, bucket) edge counts -> shared chunk schedule
    key = (core.astype(np.int64) * TILES + tl) * NB + bkt
    cnt = np.bincount(key, minlength=C * TILES * NB).reshape(C, TILES, NB)
    counts = np.ceil(cnt.max(axis=0) / 128.0).astype(np.int64)  # [TILES, NB]

    sched = []  # per group: per bucket: dict(slot_ofs, nslot, tiles=[(t, nch_t)])
    ofs = 0
    for g in range(NGRP):
        buckets = []
        for b in range(NB):
            tl_list = []
            s0 = ofs
            for t in range(g * TGRP, (g + 1) * TGRP):
                nch_t = int(counts[t, b])
                tl_list.append((t, nch_t))
                ofs += nch_t * 128
            buckets.append(dict(slot_ofs=s0, nslot=ofs - s0, tiles=tl_list))
        sched.append(buckets)
    NSLOT = ofs
    NCH = NSLOT // 128

    # slot offset of each (t, b) block within the stream (same for all cores)
    block_ofs = np.zeros((TILES, NB), dtype=np.int64)
    for g in range(NGRP):
        for b in range(NB):
            blk = sched[g][b]
            o = blk['slot_ofs']
            for (t, nch_t) in blk['tiles']:
                block_ofs[t, b] = o
                o += nch_t * 128

    # order edges by (core, g, b, t); within (t,b) order irrelevant
    order = np.lexsort((tl, bkt, (tl // TGRP), core))
    idxv_s = idxv[order]
    pdst_s = pdst[order]
    key_s = key[order]
    core_s = core[order]

    idx_arrs, dst_arrs = [], []
    for c in range(C):
        idx_c = np.zeros(NSLOT, dtype=np.int16)
        dst_c = np.full(NSLOT, -1.0, dtype=np.float32)
        mask = core_s == c
        iv = idxv_s[mask]
        pv = pdst_s[mask]
        kv = key_s[mask] - (c * TILES) * NB  # tl*NB + bkt
        t_arr = kv // NB
        b_arr = kv % NB
        if len(kv):
            new_blk = np.empty(len(kv), dtype=bool)
            new_blk[0] = True
            new_blk[1:] = kv[1:] != kv[:-1]
            blk_start = np.flatnonzero(new_blk)
            blk_len = np.diff(np.append(blk_start, len(kv)))
            within = np.arange(len(kv)) - np.repeat(blk_start, blk_len)
            slot = block_ofs[t_arr, b_arr] + within
            idx_c[slot] = iv.astype(np.int16)
            dst_c[slot] = pv.astype(np.float32)
        idx_w = np.tile(idx_c.reshape(NSLOT // 16, 16).T, (8, 1)).copy()
        dst_w = dst_c.reshape(NCH, 128).T.astype(np.float16).copy()
        idx_arrs.append(idx_w)
        dst_arrs.append(dst_w)

    dinv = (d ** -0.5).astype(np.float32)
    meta = dict(NLOC=NLOC, TILES=TILES, NPAD=NPAD, NB=NB, NSLOT=NSLOT, NCH=NCH,
                TGRP=TGRP, NGRP=NGRP, sched=sched, bucket_rows=bucket_rows)
    vecs = dict(dinv=dinv, dinv2=(1.0 / d).astype(np.float32),
                sqrtd=(d ** 0.5).astype(np.float32))
    return meta, vecs, idx_arrs, dst_arrs


def _col_layout(vec_loc, TILES, NPAD, pad_val):
    """[NLOC] -> padded [128, TILES] SBUF layout (node t*128+p -> [p, t])."""
    v = np.full(NPAD, pad_val, dtype=np.float32)
    v[:len(vec_loc)] = vec_loc
    return v.reshape(TILES, 128).T.copy()


def _build_program(C, meta, thetas):
    NLOC, TILES = meta['NLOC'], meta['TILES']
    NPAD, NB, NSLOT, NCH = meta['NPAD'], meta['NB'], meta['NSLOT'], meta['NCH']
    TGRP, NGRP, sched = meta['TGRP'], meta['NGRP'], meta['sched']
    bucket_rows = meta['bucket_rows']
    NPADG = NPAD * C  # padded global rows

    nc = bacc.Bacc('TRN2', target_bir_lowering=False, debug=False,
                   enable_asserts=False, num_devices=C, num_swdge_queues=4,
                   dynamic_dma_scratch_size=32768)

    fp16 = mybir.dt.float16
    featT_in = nc.dram_tensor('featT', [FIN, NPAD], fp16, kind='ExternalInput')
    W1_in = nc.dram_tensor('W1', [FIN, F], fp16, kind='ExternalInput')
    b1_in = nc.dram_tensor('b1', [1, F], fp16, kind='ExternalInput')
    idx_in = nc.dram_tensor('idx16', [128, NSLOT // 16], i16, kind='ExternalInput')
    dstloc_in = nc.dram_tensor('dstloc', [128, NCH], fp16, kind='ExternalInput')
    iota_in = nc.dram_tensor('iota', [128, 128], fp16, kind='ExternalInput')
    dinv_in = nc.dram_tensor('dinv', [128, TILES], f32, kind='ExternalInput')
    dinv2n_in = nc.dram_tensor('dinv2n', [128, TILES], f32, kind='ExternalInput')
    sqrtd_in = nc.dram_tensor('sqrtd', [128, TILES], f32, kind='ExternalInput')

    out_t = nc.dram_tensor('out', [NLOC, NFILT * F], f32, kind='ExternalOutput')

    bounce = nc.dram_tensor('bounce', [NPAD, 2 * F], fp16, kind='Internal')
    g2_space = 'Shared' if C > 4 else 'Local'
    g2full = [nc.dram_tensor(f'g2full{k}', [NPADG, 2 * F], fp16, kind='Internal',
                             addr_space=g2_space) for k in range(KPOW)]
    # local double-buffered copy of the gathered table: random 256B gather
    # reads against the Shared table pace at remote-HBM latency; a bulk
    # sequential copy to Local HBM first makes the gathers local.
    tblL = [nc.dram_tensor(f'tblL{j}', [NPADG, 2 * F], fp16, kind='Internal')
            for j in range(2)]
    psnap = {k: nc.dram_tensor(f'psnap{k}', [NPAD, F], f32, kind='Internal')
             for k in range(2, KPOW + 1)}

    groups_all = [list(range(C))]

    with tile.TileContext(nc) as tc:
        with tc.tile_pool(name='resident', bufs=1) as res:
            idx_sb = res.tile([128, NSLOT // 16], i16)
            nc.sync.dma_start(idx_sb[:], idx_in[:])
            dstloc_sb = res.tile([128, NCH], fp16)
            nc.sync.dma_start(dstloc_sb[:], dstloc_in[:])
            iota_sb = res.tile([128, 128], fp16)
            nc.sync.dma_start(iota_sb[:], iota_in[:])
            W1_sb = res.tile([FIN, F], fp16)
            nc.sync.dma_start(W1_sb[:], W1_in[:])
            b1_sb = res.tile([1, F], fp16)
            nc.sync.dma_start(b1_sb[:], b1_in[:])
            ones_sb = res.tile([1, 128], fp16)
            nc.vector.memset(ones_sb[:], 1.0)
            zero_sb = res.tile([128, F], f32)
            nc.vector.memset(zero_sb[:], 0.0)
            dinv_sb = res.tile([128, TILES], f32)
            nc.sync.dma_start(dinv_sb[:], dinv_in[:])
            dinv2n_sb = res.tile([128, TILES], f32)
            nc.sync.dma_start(dinv2n_sb[:], dinv2n_in[:])
            sqrtd_sb = res.tile([128, TILES], f32)
            nc.sync.dma_start(sqrtd_sb[:], sqrtd_in[:])
            g_loc = res.tile([128, TILES, F], f32)

            # ---------- phase 0: h = lrelu(x @ W1 + b1); g_0 = dinv * h ----
            with tc.tile_pool(name='h_sb', bufs=3) as hp, \
                 tc.tile_pool(name='h_ps', bufs=3, space='PSUM') as hps, \
                 tc.tile_pool(name='xfer0', bufs=2) as xfer:
                ActF = mybir.ActivationFunctionType
                for g in range(NGRP):
                    hbuf = xfer.tile([128, TGRP, F], f32, tag='hbuf')
                    g2b = xfer.tile([128, TGRP, 2 * F], fp16, tag='g2b')
                    for ti in range(TGRP):
                        t = g * TGRP + ti
                        ft = hp.tile([FIN, 128], fp16, tag='ft')
                        nc.sync.dma_start(ft[:], featT_in[:, t * 128:(t + 1) * 128])
                        hps_t = hps.tile([128, F], f32, tag='hps')
                        nc.tensor.matmul(hps_t[:], ft[:], W1_sb[:], start=True, stop=False)
                        nc.tensor.matmul(hps_t[:], ones_sb[:], b1_sb[:], start=False, stop=True)
                        # h = leaky_relu(z)  (= p_0 snapshot, since sqrtd*dinv=1)
                        nc.scalar.activation(hbuf[:, ti, :], hps_t[:],
                                             ActF.Lrelu, alpha=0.01)
                    for ti in range(TGRP):
                        t = g * TGRP + ti
                        # g_0 = dinv * h
                        nc.scalar.activation(g_loc[:, t, :], hbuf[:, ti, :],
                                             ActF.Identity,
                                             scale=dinv_sb[:, t:t + 1])
                    for ti in range(TGRP):
                        t = g * TGRP + ti
                        # exchange row = fp16(g) in cols 0:F (junk in F:2F)
                        nc.scalar.activation(g2b[:, ti, 0:F], g_loc[:, t, :],
                                             ActF.Copy)
                    r0 = g * TGRP * 128
                    nc.sync.dma_start(
                        bounce.ap()[r0:r0 + TGRP * 128, :].rearrange(
                            '(t p) f -> p t f', p=128), g2b[:])
            nc.gpsimd.collective_compute(
                'AllGather', mybir.AluOpType.bypass, replica_groups=groups_all,
                ins=[bounce.ap().opt()], outs=[g2full[0].ap().opt()])
            NBG = (NPADG + bucket_rows - 1) // bucket_rows
            for b in range(NBG):
                lo_r = b * bucket_rows
                hi_r = min(lo_r + bucket_rows, NPADG)
                nc.sync.dma_start(tblL[0].ap()[lo_r:hi_r, :],
                                  g2full[0].ap()[lo_r:hi_r, :])

            # ---------- rounds 1..KPOW ----------
            gcount = [0]  # program-global SWDGE gather counter: Tile assigns
            # DMASW lanes round-robin (8 lanes) over Pool DMA insts in order;
            # queue = i % 4 keeps each lane on one queue (lanes 8 = 2x queues 4)
            for k in range(1, KPOW + 1):
                src_full = tblL[(k - 1) % 2].ap()
                # size msg/oh buffer counts to the SBUF budget
                nch_max = max(blk['nslot'] // 128 for gg in sched for blk in gg
                              if blk['nslot'] > 0)
                PIECE = int(os.environ.get('BW_PIECE', '34'))  # chunks per gather piece
                oh_pp = nch_max * 256      # per-partition bytes of one oh tile
                msg_pp = PIECE * 256       # per-partition bytes of one msg piece
                res_pp = (NSLOT // 16 * 2 + NCH * 2 + TILES * F * 4 +
                          3 * TILES * 4 + 2048 +
                          2 * (TGRP * F * 4 + TGRP * 2 * F * 2))
                budget_pp = 174 * 1024 - res_pp
                oh_bufs = 4
                msg_bufs = int(max(3, min(24, (budget_pp - oh_bufs * oh_pp - 8192)
                                          // msg_pp)))
                with tc.tile_pool(name=f'msg{k}', bufs=msg_bufs) as msgp, \
                     tc.tile_pool(name=f'oh{k}', bufs=oh_bufs) as ohp, \
                     tc.tile_pool(name=f'ps{k}', bufs=8, space='PSUM') as psp, \
                     tc.tile_pool(name=f'xf{k}', bufs=2) as xfer:
                    for g in range(NGRP):
                        msgs, ohs = [], []
                        for b in range(NB):
                            blk = sched[g][b]
                            nsl, s0 = blk['nslot'], blk['slot_ofs']
                            if nsl == 0:
                                msgs.append(None)
                                ohs.append(None)
                                continue
                            nch = nsl // 128
                            b_lo = b * bucket_rows
                            b_hi = min(b_lo + bucket_rows, NPADG)
                            pieces = []  # (c_lo, c_hi, tile)
                            for c_lo in range(0, nch, PIECE):
                                c_hi = min(c_lo + PIECE, nch)
                                npc = c_hi - c_lo
                                m = msgp.tile([128, PIECE, 2 * F], fp16, tag='msg')
                                qn = gcount[0] % 4
                                gcount[0] += 1
                                p0 = s0 + c_lo * 128
                                p1 = s0 + c_hi * 128
                                nc.gpsimd.dma_gather(
                                    m[:, 0:npc, :], src_full[b_lo:b_hi, :],
                                    idx_sb[:, p0 // 16:p1 // 16],
                                    num_idxs=npc * 128, num_idxs_reg=npc * 128,
                                    elem_size=2 * F, single_packet=False,
                                    queue_num=qn)
                                pieces.append((c_lo, c_hi, m))
                            oh = ohp.tile([128, nch, 128], fp16, tag='oh')
                            c0 = s0 // 128
                            nc.vector.tensor_tensor(
                                oh[:],
                                iota_sb[:, None, :].broadcast_to([128, nch, 128]),
                                dstloc_sb[:, c0:c0 + nch, None].broadcast_to(
                                    [128, nch, 128]),
                                AluOpType.is_equal)
                            msgs.append(pieces)
                            ohs.append(oh)
                        gsnap = xfer.tile([128, TGRP, F], f32, tag='gsnap')
                        g2b = xfer.tile([128, TGRP, 2 * F], fp16, tag='g2b')
                        for ti in range(TGRP):
                            t = g * TGRP + ti
                            mm = []
                            for b in range(NB):
                                blk = sched[g][b]
                                o = 0
                                for (tt, nch_t) in blk['tiles']:
                                    if tt == t and nch_t > 0:
                                        mm += [(b, o + j) for j in range(nch_t)]
                                    o += nch_t
                            if mm:
                                ps = psp.tile([128, F], f32, tag='ps')
                                for q, (b, j) in enumerate(mm):
                                    for (c_lo, c_hi, m) in msgs[b]:
                                        if c_lo <= j < c_hi:
                                            break
                                    nc.tensor.matmul(
                                        ps[:], ohs[b][:, j, :],
                                        m[:, j - c_lo, 0:F],
                                        start=(q == 0), stop=(q == len(mm) - 1))
                                # g -= dinv2 * agg
                                nc.vector.scalar_tensor_tensor(
                                    g_loc[:, t, :], ps[:], dinv2n_sb[:, t:t + 1],
                                    g_loc[:, t, :], op0=AluOpType.mult, op1=AluOpType.add)
                        ActF = mybir.ActivationFunctionType
                        if k >= 2:
                            for ti in range(TGRP):
                                t = g * TGRP + ti
                                # p_k = sqrtd * g  (on the idle ACT engine)
                                nc.scalar.activation(
                                    gsnap[:, ti, :], g_loc[:, t, :],
                                    ActF.Identity, scale=sqrtd_sb[:, t:t + 1])
                        if k < KPOW:
                            for ti in range(TGRP):
                                t = g * TGRP + ti
                                nc.scalar.activation(g2b[:, ti, 0:F],
                                                     g_loc[:, t, :], ActF.Copy)
                        r0 = g * TGRP * 128
                        if k >= 2:
                            nc.sync.dma_start(
                                psnap[k].ap()[r0:r0 + TGRP * 128, :].rearrange(
                                    '(t p) f -> p t f', p=128), gsnap[:])
                        if k < KPOW:
                            nc.sync.dma_start(
                                bounce.ap()[r0:r0 + TGRP * 128, :].rearrange(
                                    '(t p) f -> p t f', p=128), g2b[:])
                if k < KPOW:
                    nc.gpsimd.collective_compute(
                        'AllGather', mybir.AluOpType.bypass, replica_groups=groups_all,
                        ins=[bounce.ap().opt()], outs=[g2full[k].ap().opt()])
                    for b in range(NBG):
                        lo_r = b * bucket_rows
                        hi_r = min(lo_r + bucket_rows, NPADG)
                        nc.sync.dma_start(tblL[k % 2].ap()[lo_r:hi_r, :],
                                          g2full[k].ap()[lo_r:hi_r, :])

            # ---------- output phase ----------
            CH = min(14, TILES)
            NOCH = (TILES + CH - 1) // CH
            with tc.tile_pool(name='pk', bufs=2 * (KPOW - 1)) as pkp, \
                 tc.tile_pool(name='acc', bufs=2) as accp:
                for ch in range(NOCH):
                    t0 = ch * CH
                    nt = min(CH, TILES - t0)
                    pks = {}
                    for k in range(2, KPOW + 1):
                        pk_t = pkp.tile([128, nt, F], f32, tag='pk')
                        nc.sync.dma_start(
                            pk_t[:], psnap[k].ap()[t0 * 128:(t0 + nt) * 128, :]
                            .rearrange('(t p) f -> p t f', p=128))
                        pks[k] = pk_t
                    ob = accp.tile([128, nt, NFILT * F], f32, tag='ob')
                    for i in range(NFILT):
                        k0 = i + 2
                        acc = ob[:, :, i * F:(i + 1) * F]
                        nc.vector.scalar_tensor_tensor(
                            acc, pks[k0][:], float(thetas[i][k0]),
                            zero_sb[:, None, :].broadcast_to([128, nt, F]),
                            op0=AluOpType.mult, op1=AluOpType.add)
                        for k in range(k0 + 1, KPOW + 1):
                            nc.vector.scalar_tensor_tensor(
                                acc, pks[k][:], float(thetas[i][k]), acc,
                                op0=AluOpType.mult, op1=AluOpType.add)
                    full_t = nt
                    while (t0 + full_t) * 128 > NLOC:
                        full_t -= 1
                    if full_t > 0:
                        nc.sync.dma_start(
                            out_t.ap()[t0 * 128:(t0 + full_t) * 128, :]
                            .rearrange('(t p) f -> p t f', p=128),
                            ob[:, 0:full_t, :])
                    if full_t < nt:
                        rem = NLOC - (t0 + full_t) * 128
                        if rem > 0:
                            nc.sync.dma_start(
                                out_t.ap()[(t0 + full_t) * 128:NLOC, :],
                                ob[0:rem, full_t, :])
    nc.compile()
    return nc


def build_in_maps(feature, W1, b1, meta, vecs, idx_arrs, dst_arrs, C):
    NLOC, TILES, NPAD = meta['NLOC'], meta['TILES'], meta['NPAD']
    iota = np.tile(np.arange(128, dtype=np.float32), (128, 1)).astype(np.float16)
    in_maps = []
    for c in range(C):
        lo, hi = c * NLOC, (c + 1) * NLOC
        featT = np.zeros((FIN, NPAD), dtype=np.float16)
        featT[:, :NLOC] = feature[lo:hi].T.astype(np.float16)
        in_maps.append({
            'featT': featT,
            'W1': W1.astype(np.float16),
            'b1': b1.reshape(1, F).astype(np.float16),
            'idx16': idx_arrs[c],
            'dstloc': dst_arrs[c],
            'iota': iota,
            'dinv': _col_layout(vecs['dinv'][lo:hi], TILES, NPAD, 1.0),
            'dinv2n': _col_layout(-vecs['dinv2'][lo:hi], TILES, NPAD, -1.0),
            'sqrtd': _col_layout(vecs['sqrtd'][lo:hi], TILES, NPAD, 1.0),
        })
    return in_maps


def run(feature, src, dst, W1, b1, C=8, bucket_rows=32768, **spmd_kwargs):
    feature = np.asarray(feature, dtype=np.float32)
    src = np.asarray(src).astype(np.int64)
    dst = np.asarray(dst).astype(np.int64)
    W1 = np.asarray(W1, dtype=np.float32)
    b1 = np.asarray(b1, dtype=np.float32)
    N = feature.shape[0]
    assert N % C == 0
    thetas = _thetas()
    meta, vecs, idx_arrs, dst_arrs = _preprocess(src, dst, N, C, bucket_rows)
    nc = _build_program(C, meta, thetas)
    in_maps = build_in_maps(feature, W1, b1, meta, vecs, idx_arrs, dst_arrs, C)
    res = bass_utils.run_bass_kernel_spmd(nc, in_maps, core_ids=list(range(C)),
                                          **spmd_kwargs)
    out = np.concatenate([res.results[c]['out'] for c in range(C)], axis=0)
    return out.astype(np.float32), res


def kernel(**inputs):
    out, _ = run(inputs['feature'], inputs['src'], inputs['dst'],
                 inputs['W1'], inputs['b1'])
    return out



# revision 15
# speedup vs baseline: 1.1745x; 1.1745x over previous
"""BWGNN (beta-wavelet GNN) Trainium2 kernel, 8-core SPMD.

Math: out_i = sqrt(d) * sum_k theta[i][k] * g_k, where g_0 = d^-1/2 * h,
g_{k+1} = g_k - d^-1 * segsum_dst(g_k[src]), h = leaky_relu(x @ W1 + b1).
All 5 filters are polynomials of the same propagation, so only 6 SpMM
rounds are needed (vs 30 in the naive formulation).

Sharding: nodes are block-sharded across 8 cores by dst. Each round a core
gathers source rows from an AllGather'd full table (rows packed as
[bf16(hi) | bf16(x-hi)] = 256B so one 256B dma_gather element carries
~f32 precision), segment-sums per 128-dst tile via one-hot matmuls on the
tensor engine (f32 PSUM accumulation), and updates its f32 state.
"""
import os
import sys
from math import comb, gamma

import numpy as np
import ml_dtypes

for _p in ('/opt/trn_rl_repo', os.path.expanduser('~/.axon_site/_ro/trn_rl_repo')):
    if os.path.isdir(_p) and _p not in sys.path:
        sys.path.append(_p)

import concourse.bass as bass
import concourse.bacc as bacc
import concourse.tile as tile
from concourse import bass_utils, mybir
from concourse.alu_op_type import AluOpType

F = 64          # hidden feature dim
FIN = 128       # input feature dim
KPOW = 6        # propagation rounds (powers 1..6; 7 snapshots incl. g_0)
NFILT = 5

f32 = mybir.dt.float32
bf16 = mybir.dt.bfloat16
i16 = mybir.dt.int16


def _thetas(d=4):
    c, off = 1.4, 2
    th = []
    for i in range(off, d + 1 + off):
        B = gamma(i + 1) * gamma(d + 1 - i + off) / gamma(d + 2 + off)
        m = d - i + off
        coeff = np.zeros(d + 1 + off, dtype=np.float64)
        for j in range(m + 1):
            coeff[i + j] = comb(m, j) * ((-1.0) ** j) / (c ** (i + j)) / (c * B)
        th.append(coeff)
    return np.array(th)  # [5, 7]


def _preprocess(src, dst, N, C, bucket_rows):
    """Host-side graph preprocessing -> per-core index tensors + schedule."""
    NLOC = N // C
    TILES = (NLOC + 127) // 128
    NPAD = TILES * 128
    TGRP = next(c for c in (7, 8, 6, 5, 4, 3, 2, 1) if TILES % c == 0)
    NGRP = TILES // TGRP

    deg = np.bincount(src, minlength=N).astype(np.float32)
    d = np.maximum(deg, 1.0)

    pos = (src // NLOC) * NPAD + (src % NLOC)       # padded global row of src
    bkt = pos // bucket_rows
    idxv = pos % bucket_rows
    NB = int((NPAD * C + bucket_rows - 1) // bucket_rows)
    assert bucket_rows <= 32768

    core = dst // NLOC
    dl = dst % NLOC
    tl = dl // 128
    pdst = dl % 128

    # per (core, tile, bucket) edge counts -> shared chunk schedule
    key = (core.astype(np.int64) * TILES + tl) * NB + bkt
    cnt = np.bincount(key, minlength=C * TILES * NB).reshape(C, TILES, NB)
    counts = np.ceil(cnt.max(axis=0) / 128.0).astype(np.int64)  # [TILES, NB]

    sched = []  # per group: per bucket: dict(slot_ofs, nslot, tiles=[(t, nch_t)])
    ofs = 0
    for g in range(NGRP):
        buckets = []
        for b in range(NB):
            tl_list = []
            s0 = ofs
            for t in range(g * TGRP, (g + 1) * TGRP):
                nch_t = int(counts[t, b])
                tl_list.append((t, nch_t))
                ofs += nch_t * 128
            buckets.append(dict(slot_ofs=s0, nslot=ofs - s0, tiles=tl_list))
        sched.append(buckets)
    NSLOT = ofs
    NCH = NSLOT // 128

    # slot offset of each (t, b) block within the stream (same for all cores)
    block_ofs = np.zeros((TILES, NB), dtype=np.int64)
    for g in range(NGRP):
        for b in range(NB):
            blk = sched[g][b]
            o = blk['slot_ofs']
            for (t, nch_t) in blk['tiles']:
                block_ofs[t, b] = o
                o += nch_t * 128

    # order edges by (core, g, b, t); within (t,b) order irrelevant
    order = np.lexsort((tl, bkt, (tl // TGRP), core))
    idxv_s = idxv[order]
    pdst_s = pdst[order]
    key_s = key[order]
    core_s = core[order]

    idx_arrs, dst_arrs = [], []
    for c in range(C):
        idx_c = np.zeros(NSLOT, dtype=np.int16)
        dst_c = np.full(NSLOT, -1.0, dtype=np.float32)
        mask = core_s == c
        iv = idxv_s[mask]
        pv = pdst_s[mask]
        kv = key_s[mask] - (c * TILES) * NB  # tl*NB + bkt
        t_arr = kv // NB
        b_arr = kv % NB
        if len(kv):
            new_blk = np.empty(len(kv), dtype=bool)
            new_blk[0] = True
            new_blk[1:] = kv[1:] != kv[:-1]
            blk_start = np.flatnonzero(new_blk)
            blk_len = np.diff(np.append(blk_start, len(kv)))
            within = np.arange(len(kv)) - np.repeat(blk_start, blk_len)
            slot = block_ofs[t_arr, b_arr] + within
            idx_c[slot] = iv.astype(np.int16)
            dst_c[slot] = pv.astype(np.float32)
        idx_w = np.tile(idx_c.reshape(NSLOT // 16, 16).T, (8, 1)).copy()
        dst_w = dst_c.reshape(NCH, 128).T.astype(np.float16).copy()
        idx_arrs.append(idx_w)
        dst_arrs.append(dst_w)

    dinv = (d ** -0.5).astype(np.float32)
    meta = dict(NLOC=NLOC, TILES=TILES, NPAD=NPAD, NB=NB, NSLOT=NSLOT, NCH=NCH,
                TGRP=TGRP, NGRP=NGRP, sched=sched, bucket_rows=bucket_rows)
    vecs = dict(dinv=dinv, dinv2=(1.0 / d).astype(np.float32),
                sqrtd=(d ** 0.5).astype(np.float32))
    return meta, vecs, idx_arrs, dst_arrs


def _col_layout(vec_loc, TILES, NPAD, pad_val):
    """[NLOC] -> padded [128, TILES] SBUF layout (node t*128+p -> [p, t])."""
    v = np.full(NPAD, pad_val, dtype=np.float32)
    v[:len(vec_loc)] = vec_loc
    return v.reshape(TILES, 128).T.copy()


def _build_program(C, meta, thetas):
    NLOC, TILES = meta['NLOC'], meta['TILES']
    NPAD, NB, NSLOT, NCH = meta['NPAD'], meta['NB'], meta['NSLOT'], meta['NCH']
    TGRP, NGRP, sched = meta['TGRP'], meta['NGRP'], meta['sched']
    bucket_rows = meta['bucket_rows']
    NPADG = NPAD * C  # padded global rows

    nc = bacc.Bacc('TRN2', target_bir_lowering=False, debug=False,
                   enable_asserts=False, num_devices=C, num_swdge_queues=4,
                   dynamic_dma_scratch_size=32768)

    fp16 = mybir.dt.float16
    featT_in = nc.dram_tensor('featT', [FIN, NPAD], fp16, kind='ExternalInput')
    W1_in = nc.dram_tensor('W1', [FIN, F], fp16, kind='ExternalInput')
    b1_in = nc.dram_tensor('b1', [1, F], fp16, kind='ExternalInput')
    idx_in = nc.dram_tensor('idx16', [128, NSLOT // 16], i16, kind='ExternalInput')
    dstloc_in = nc.dram_tensor('dstloc', [128, NCH], fp16, kind='ExternalInput')
    iota_in = nc.dram_tensor('iota', [128, 128], fp16, kind='ExternalInput')
    dinv_in = nc.dram_tensor('dinv', [128, TILES], f32, kind='ExternalInput')
    dinv2n_in = nc.dram_tensor('dinv2n', [128, TILES], f32, kind='ExternalInput')
    sqrtd_in = nc.dram_tensor('sqrtd', [128, TILES], f32, kind='ExternalInput')

    out_t = nc.dram_tensor('out', [NLOC, NFILT * F], f32, kind='ExternalOutput')

    bounce = nc.dram_tensor('bounce', [NPAD, 2 * F], fp16, kind='Internal')
    g2_space = 'Shared' if C > 4 else 'Local'
    g2full = [nc.dram_tensor(f'g2full{k}', [NPADG, 2 * F], fp16, kind='Internal',
                             addr_space=g2_space) for k in range(KPOW)]
    # local double-buffered copy of the gathered table: random 256B gather
    # reads against the Shared table pace at remote-HBM latency; a bulk
    # sequential copy to Local HBM first makes the gathers local.
    tblL = [nc.dram_tensor(f'tblL{j}', [NPADG, 2 * F], fp16, kind='Internal')
            for j in range(2)]
    psnap = {k: nc.dram_tensor(f'psnap{k}', [NPAD, F], f32, kind='Internal')
             for k in range(2, KPOW + 1)}

    groups_all = [list(range(C))]

    with tile.TileContext(nc) as tc:
        with tc.tile_pool(name='resident', bufs=1) as res:
            idx_sb = res.tile([128, NSLOT // 16], i16)
            nc.sync.dma_start(idx_sb[:], idx_in[:])
            dstloc_sb = res.tile([128, NCH], fp16)
            nc.sync.dma_start(dstloc_sb[:], dstloc_in[:])
            iota_sb = res.tile([128, 128], fp16)
            nc.sync.dma_start(iota_sb[:], iota_in[:])
            W1_sb = res.tile([FIN, F], fp16)
            nc.sync.dma_start(W1_sb[:], W1_in[:])
            b1_sb = res.tile([1, F], fp16)
            nc.sync.dma_start(b1_sb[:], b1_in[:])
            ones_sb = res.tile([1, 128], fp16)
            nc.vector.memset(ones_sb[:], 1.0)
            zero_sb = res.tile([128, F], f32)
            nc.vector.memset(zero_sb[:], 0.0)
            dinv_sb = res.tile([128, TILES], f32)
            nc.sync.dma_start(dinv_sb[:], dinv_in[:])
            dinv2n_sb = res.tile([128, TILES], f32)
            nc.sync.dma_start(dinv2n_sb[:], dinv2n_in[:])
            sqrtd_sb = res.tile([128, TILES], f32)
            nc.sync.dma_start(sqrtd_sb[:], sqrtd_in[:])
            g_loc = res.tile([128, TILES, F], f32)

            # ---------- phase 0: h = lrelu(x @ W1 + b1); g_0 = dinv * h ----
            with tc.tile_pool(name='h_sb', bufs=3) as hp, \
                 tc.tile_pool(name='h_ps', bufs=3, space='PSUM') as hps, \
                 tc.tile_pool(name='xfer0', bufs=2) as xfer:
                ActF = mybir.ActivationFunctionType
                for g in range(NGRP):
                    hbuf = xfer.tile([128, TGRP, F], f32, tag='hbuf')
                    g2b = xfer.tile([128, TGRP, 2 * F], fp16, tag='g2b')
                    for ti in range(TGRP):
                        t = g * TGRP + ti
                        ft = hp.tile([FIN, 128], fp16, tag='ft')
                        nc.sync.dma_start(ft[:], featT_in[:, t * 128:(t + 1) * 128])
                        hps_t = hps.tile([128, F], f32, tag='hps')
                        nc.tensor.matmul(hps_t[:], ft[:], W1_sb[:], start=True, stop=False)
                        nc.tensor.matmul(hps_t[:], ones_sb[:], b1_sb[:], start=False, stop=True)
                        # h = leaky_relu(z)  (= p_0 snapshot, since sqrtd*dinv=1)
                        nc.scalar.activation(hbuf[:, ti, :], hps_t[:],
                                             ActF.Lrelu, alpha=0.01)
                    for ti in range(TGRP):
                        t = g * TGRP + ti
                        # g_0 = dinv * h
                        nc.scalar.activation(g_loc[:, t, :], hbuf[:, ti, :],
                                             ActF.Identity,
                                             scale=dinv_sb[:, t:t + 1])
                    for ti in range(TGRP):
                        t = g * TGRP + ti
                        # exchange row = fp16(g) in cols 0:F (junk in F:2F)
                        nc.scalar.activation(g2b[:, ti, 0:F], g_loc[:, t, :],
                                             ActF.Copy)
                    r0 = g * TGRP * 128
                    nc.sync.dma_start(
                        bounce.ap()[r0:r0 + TGRP * 128, :].rearrange(
                            '(t p) f -> p t f', p=128), g2b[:])
            nc.gpsimd.collective_compute(
                'AllGather', mybir.AluOpType.bypass, replica_groups=groups_all,
                ins=[bounce.ap().opt()], outs=[g2full[0].ap().opt()])
            NBG = (NPADG + bucket_rows - 1) // bucket_rows
            for b in range(NBG):
                lo_r = b * bucket_rows
                hi_r = min(lo_r + bucket_rows, NPADG)
                nc.sync.dma_start(tblL[0].ap()[lo_r:hi_r, :],
                                  g2full[0].ap()[lo_r:hi_r, :])

            # ---------- rounds 1..KPOW ----------
            gcount = [0]  # program-global SWDGE gather counter: Tile assigns
            # DMASW lanes round-robin (8 lanes) over Pool DMA insts in order;
            # queue = i % 4 keeps each lane on one queue (lanes 8 = 2x queues 4)
            for k in range(1, KPOW + 1):
                src_full = tblL[(k - 1) % 2].ap()
                # size msg/oh buffer counts to the SBUF budget
                nch_max = max(blk['nslot'] // 128 for gg in sched for blk in gg
                              if blk['nslot'] > 0)
                PIECE = int(os.environ.get('BW_PIECE', '24'))  # chunks per gather piece
                oh_pp = nch_max * 256      # per-partition bytes of one oh tile
                msg_pp = PIECE * 256       # per-partition bytes of one msg piece
                res_pp = (NSLOT // 16 * 2 + NCH * 2 + TILES * F * 4 +
                          3 * TILES * 4 + 2048 +
                          2 * (TGRP * F * 4 + TGRP * 2 * F * 2))
                budget_pp = 174 * 1024 - res_pp
                oh_bufs = 6
                msg_bufs = int(max(3, min(24, (budget_pp - oh_bufs * oh_pp)
                                          // msg_pp)))
                with tc.tile_pool(name=f'msg{k}', bufs=msg_bufs) as msgp, \
                     tc.tile_pool(name=f'oh{k}', bufs=oh_bufs) as ohp, \
                     tc.tile_pool(name=f'ps{k}', bufs=8, space='PSUM') as psp, \
                     tc.tile_pool(name=f'xf{k}', bufs=2) as xfer:
                    for g in range(NGRP):
                        msgs, ohs = [], []
                        for b in range(NB):
                            blk = sched[g][b]
                            nsl, s0 = blk['nslot'], blk['slot_ofs']
                            if nsl == 0:
                                msgs.append(None)
                                ohs.append(None)
                                continue
                            nch = nsl // 128
                            b_lo = b * bucket_rows
                            b_hi = min(b_lo + bucket_rows, NPADG)
                            pieces = []  # (c_lo, c_hi, tile)
                            for c_lo in range(0, nch, PIECE):
                                c_hi = min(c_lo + PIECE, nch)
                                npc = c_hi - c_lo
                                m = msgp.tile([128, PIECE, 2 * F], fp16, tag='msg')
                                qn = gcount[0] % 4
                                gcount[0] += 1
                                p0 = s0 + c_lo * 128
                                p1 = s0 + c_hi * 128
                                nc.gpsimd.dma_gather(
                                    m[:, 0:npc, :], src_full[b_lo:b_hi, :],
                                    idx_sb[:, p0 // 16:p1 // 16],
                                    num_idxs=npc * 128, num_idxs_reg=npc * 128,
                                    elem_size=2 * F, single_packet=False,
                                    queue_num=qn)
                                pieces.append((c_lo, c_hi, m))
                            oh = ohp.tile([128, nch, 128], fp16, tag='oh')
                            c0 = s0 // 128
                            nc.vector.tensor_tensor(
                                oh[:],
                                iota_sb[:, None, :].broadcast_to([128, nch, 128]),
                                dstloc_sb[:, c0:c0 + nch, None].broadcast_to(
                                    [128, nch, 128]),
                                AluOpType.is_equal)
                            msgs.append(pieces)
                            ohs.append(oh)
                        gsnap = xfer.tile([128, TGRP, F], f32, tag='gsnap')
                        g2b = xfer.tile([128, TGRP, 2 * F], fp16, tag='g2b')
                        for ti in range(TGRP):
                            t = g * TGRP + ti
                            mm = []
                            for b in range(NB):
                                blk = sched[g][b]
                                o = 0
                                for (tt, nch_t) in blk['tiles']:
                                    if tt == t and nch_t > 0:
                                        mm += [(b, o + j) for j in range(nch_t)]
                                    o += nch_t
                            if mm:
                                ps = psp.tile([128, F], f32, tag='ps')
                                for q, (b, j) in enumerate(mm):
                                    for (c_lo, c_hi, m) in msgs[b]:
                                        if c_lo <= j < c_hi:
                                            break
                                    nc.tensor.matmul(
                                        ps[:], ohs[b][:, j, :],
                                        m[:, j - c_lo, 0:F],
                                        start=(q == 0), stop=(q == len(mm) - 1))
                                # g -= dinv2 * agg
                                nc.vector.scalar_tensor_tensor(
                                    g_loc[:, t, :], ps[:], dinv2n_sb[:, t:t + 1],
                                    g_loc[:, t, :], op0=AluOpType.mult, op1=AluOpType.add)
                        ActF = mybir.ActivationFunctionType
                        if k >= 2:
                            for ti in range(TGRP):
                                t = g * TGRP + ti
                                # p_k = sqrtd * g  (on the idle ACT engine)
                                nc.scalar.activation(
                                    gsnap[:, ti, :], g_loc[:, t, :],
                                    ActF.Identity, scale=sqrtd_sb[:, t:t + 1])
                        if k < KPOW:
                            for ti in range(TGRP):
                                t = g * TGRP + ti
                                nc.scalar.activation(g2b[:, ti, 0:F],
                                                     g_loc[:, t, :], ActF.Copy)
                        r0 = g * TGRP * 128
                        if k >= 2:
                            nc.sync.dma_start(
                                psnap[k].ap()[r0:r0 + TGRP * 128, :].rearrange(
                                    '(t p) f -> p t f', p=128), gsnap[:])
                        if k < KPOW:
                            nc.sync.dma_start(
                                bounce.ap()[r0:r0 + TGRP * 128, :].rearrange(
                                    '(t p) f -> p t f', p=128), g2b[:])
                if k < KPOW:
                    nc.gpsimd.collective_compute(
                        'AllGather', mybir.AluOpType.bypass, replica_groups=groups_all,
                        ins=[bounce.ap().opt()], outs=[g2full[k].ap().opt()])
                    for b in range(NBG):
                        lo_r = b * bucket_rows
                        hi_r = min(lo_r + bucket_rows, NPADG)
                        nc.sync.dma_start(tblL[k % 2].ap()[lo_r:hi_r, :],
                                          g2full[k].ap()[lo_r:hi_r, :])

            # ---------- output phase ----------
            CH = min(14, TILES)
            NOCH = (TILES + CH - 1) // CH
            with tc.tile_pool(name='pk', bufs=2 * (KPOW - 1)) as pkp, \
                 tc.tile_pool(name='acc', bufs=2) as accp:
                for ch in range(NOCH):
                    t0 = ch * CH
                    nt = min(CH, TILES - t0)
                    pks = {}
                    for k in range(2, KPOW + 1):
                        pk_t = pkp.tile([128, nt, F], f32, tag='pk')
                        nc.sync.dma_start(
                            pk_t[:], psnap[k].ap()[t0 * 128:(t0 + nt) * 128, :]
                            .rearrange('(t p) f -> p t f', p=128))
                        pks[k] = pk_t
                    ob = accp.tile([128, nt, NFILT * F], f32, tag='ob')
                    for i in range(NFILT):
                        k0 = i + 2
                        acc = ob[:, :, i * F:(i + 1) * F]
                        nc.vector.scalar_tensor_tensor(
                            acc, pks[k0][:], float(thetas[i][k0]),
                            zero_sb[:, None, :].broadcast_to([128, nt, F]),
                            op0=AluOpType.mult, op1=AluOpType.add)
                        for k in range(k0 + 1, KPOW + 1):
                            nc.vector.scalar_tensor_tensor(
                                acc, pks[k][:], float(thetas[i][k]), acc,
                                op0=AluOpType.mult, op1=AluOpType.add)
                    full_t = nt
                    while (t0 + full_t) * 128 > NLOC:
                        full_t -= 1
                    if full_t > 0:
                        nc.sync.dma_start(
                            out_t.ap()[t0 * 128:(t0 + full_t) * 128, :]
                            .rearrange('(t p) f -> p t f', p=128),
                            ob[:, 0:full_t, :])
                    if full_t < nt:
                        rem = NLOC - (t0 + full_t) * 128
                        if rem > 0:
                            nc.sync.dma_start(
                                out_t.ap()[(t0 + full_t) * 128:NLOC, :],
                                ob[0:rem, full_t, :])
    nc.compile()
    return nc


def build_in_maps(feature, W1, b1, meta, vecs, idx_arrs, dst_arrs, C):
    NLOC, TILES, NPAD = meta['NLOC'], meta['TILES'], meta['NPAD']
    iota = np.tile(np.arange(128, dtype=np.float32), (128, 1)).astype(np.float16)
    in_maps = []
    for c in range(C):
        lo, hi = c * NLOC, (c + 1) * NLOC
        featT = np.zeros((FIN, NPAD), dtype=np.float16)
        featT[:, :NLOC] = feature[lo:hi].T.astype(np.float16)
        in_maps.append({
            'featT': featT,
            'W1': W1.astype(np.float16),
            'b1': b1.reshape(1, F).astype(np.float16),
            'idx16': idx_arrs[c],
            'dstloc': dst_arrs[c],
            'iota': iota,
            'dinv': _col_layout(vecs['dinv'][lo:hi], TILES, NPAD, 1.0),
            'dinv2n': _col_layout(-vecs['dinv2'][lo:hi], TILES, NPAD, -1.0),
            'sqrtd': _col_layout(vecs['sqrtd'][lo:hi], TILES, NPAD, 1.0),
        })
    return in_maps


def run(feature, src, dst, W1, b1, C=8, bucket_rows=32768, **spmd_kwargs):
    feature = np.asarray(feature, dtype=np.float32)
    src = np.asarray(src).astype(np.int64)
    dst = np.asarray(dst).astype(np.int64)
    W1 = np.asarray(W1, dtype=np.float32)
    b1 = np.asarray(b1, dtype=np.float32)
    N = feature.shape[0]
    assert N % C == 0
    thetas = _thetas()
    meta, vecs, idx_arrs, dst_arrs = _preprocess(src, dst, N, C, bucket_rows)
    nc = _build_program(C, meta, thetas)
    in_maps = build_in_maps(feature, W1, b1, meta, vecs, idx_arrs, dst_arrs, C)
    res = bass_utils.run_bass_kernel_spmd(nc, in_maps, core_ids=list(range(C)),
                                          **spmd_kwargs)
    out = np.concatenate([res.results[c]['out'] for c in range(C)], axis=0)
    return out.astype(np.float32), res


def kernel(**inputs):
    out, _ = run(inputs['feature'], inputs['src'], inputs['dst'],
                 inputs['W1'], inputs['b1'])
    return out

